# revision 14
# baseline (speedup 1.0000x reference)
"""Cross-attention Trainium2 Bass kernel.

Math (per batch element b, one per NeuronCore):
    q = x Wq + bq            [Sq, 8]
    k = ctx Wk + bk          [Sk, 8]
    v = ctx Wv + bv          [Sk, 8]
    scores = q k^T           [Sq, Sk]
    w = softmax(scores)      (no max subtraction; scores are bounded ~|31|)
    out = w v                [Sq, 8]
    y = out Wo + bo          [Sq, 1024]

Layout (contractions need the contracted dim on SBUF partitions, so x/ctx
are fed pre-transposed as xT/ctxT [1024, 2048], cast to fp16 on host):
    kT[8, t], vT[8, t]  = Wkv^T ctxT
    qT[8, s]  = Wq^T xT
    v9[t, 9]  = PE-transpose of [vT; ones]   (ones row 8 -> denominator)
    E^T[t, s] = exp(kT^T qT)           (scoresT via PE, exp via ScalarE)
    oe[9, s]  = v9^T E^T  accumulated over t-chunks:
        rows 0-7 = sum_t E v   row 8 = sum_t E  (softmax denominator)
    den[s, 1] = oe^T e8  (e8 = unit vector for row 8; transposes the
        denominator into per-partition layout for the y scaling)
    y[s, d]   = (oe^T [Wo; bo]) * (1/den)  per-partition
                (den*bo/den = bo, so the bias survives the normalization).

Precision: the scores path (x, ctx, Wq, Wkv, kT, qT) runs in fp16 - exp
amplifies score error, and bf16 there costs ~1.3e-2 end-to-end while fp16
is ~8x finer.  E can reach e^31 which overflows fp16, so the weights path
(et, v9, outU, wob, y) runs in bf16; those errors are not exp-amplified.

PE array packing (the PE drops to 1.2GHz under sustained load, so the
count of lead matmuls is the budget; concurrent tiles at different
tile_positions are nearly free):
  - scores (K=8): 4-way row tiling, positions (0/32/64/96, 0); kT/qT are
    replicated at the four partition bands via SBUF-to-SBUF DMA
    (engine-free).  Each group computes 4 t-chunks into a [128, 2048]
    PSUM group (4 banks, single buffered; the next group's scores wait
    for exp to drain, which is fine because ACT is the pacer).
  - oe (M=9): 2-way column tiling, positions (0, 0/64); even t-chunks
    accumulate into PSUM rows 0-8, odd into rows 64-72; reduced per
    s-tile by DVE (stage one band to SBUF, then add).
  - y (K=9): 2-way row tiling, positions (0/64, 0); outU/wob replicated
    at bands 0/64 (outU band copy via DMA).
  - q/kv projections (M=8/40): 2-way column tiling over even/odd dc
    chunks; the two partials are combined during PSUM evacuation with
    scalar_tensor_tensor, which folds in the bias add.

Schedule: x DMA + q projection run first (they gate the first scores
group); ctx DMA / kv projection / v9 transposes stream inside the st=0
iteration so their DMA hides under the exp pipeline; y for s-tile st
overlaps the scores pipeline of st+1.  Engine balance per core: ACT =
exp (16 x [128,2048]) + 8 y-scales ~= 37us, DVE = 24 y-scales + all PSUM
evacuation ~= 37us, DMA in+out+replication ~= 37us, PE ~= 96 lead
matmuls ~= 41us at the throttled clock.
"""

import numpy as np

B = 8
SQ = 2048
SK = 2048
D = 1024
H = 8
N_CORES = 8

_CACHE: dict = {}


def _build_nc():
    import concourse.bacc as bacc
    import concourse.mybir as mybir
    from concourse.bass import ds, ts
    from concourse.tile import TileContext

    F32 = mybir.dt.float32
    BF16 = mybir.dt.bfloat16
    FP16 = mybir.dt.float16
    EXP = mybir.ActivationFunctionType.Exp
    CPY = mybir.ActivationFunctionType.Copy
    ADD = mybir.AluOpType.add

    nc = bacc.Bacc("TRN2", target_bir_lowering=False, debug=False)

    xT = nc.dram_tensor("xT", [D, SQ], FP16, kind="ExternalInput").ap()
    ctxT = nc.dram_tensor("ctxT", [D, SK], FP16, kind="ExternalInput").ap()
    # per-dc blocks, duplicated at column band 64 for 2-way col tiling
    wq_d = nc.dram_tensor("wq_l", [128, 8 * 72], FP16,
                          kind="ExternalInput").ap()
    wkv_d = nc.dram_tensor("wkv_l", [128, 8 * 104], FP16,
                           kind="ExternalInput").ap()
    bq_d = nc.dram_tensor("bq8", [8, 1], F32, kind="ExternalInput").ap()
    # bkv: bv at rows 0-7, bk at rows 32-39 (scalar_tensor_tensor needs the
    # scalar AP at the same base partition as its SBUF tensor operand)
    bkv_d = nc.dram_tensor("bkv", [128, 1], F32, kind="ExternalInput").ap()
    wob_d = nc.dram_tensor("wob", [128, D], BF16, kind="ExternalInput").ap()
    id_d = nc.dram_tensor("ident9", [9, 9], BF16, kind="ExternalInput").ap()
    vc_d = nc.dram_tensor("vconst", [1, SK], BF16, kind="ExternalInput").ap()
    e8_d = nc.dram_tensor("e8", [9, 1], BF16, kind="ExternalInput").ap()
    y_d = nc.dram_tensor("y", [SQ, D], BF16, kind="ExternalOutput").ap()

    with TileContext(nc) as tc:
        with tc.tile_pool(name="consts", bufs=1) as cp:
            wq_sb = cp.tile([128, 8 * 72], FP16)
            wkv_sb = cp.tile([128, 8 * 104], FP16)
            bq_sb = cp.tile([8, 1], F32)
            bkv_sb = cp.tile([128, 1], F32)
            wob_sb = cp.tile([128, D], BF16)   # [Wo; bo] at bands 0-8, 64-72
            id_sb = cp.tile([9, 9], BF16)
            e8_sb = cp.tile([9, 1], BF16)
            nc.sync.dma_start(wq_sb, wq_d)
            nc.sync.dma_start(wkv_sb, wkv_d)
            nc.sync.dma_start(bq_sb, bq_d)
            nc.sync.dma_start(bkv_sb, bkv_d)
            nc.sync.dma_start(wob_sb, wob_d)
            nc.sync.dma_start(id_sb, id_d)
            nc.sync.dma_start(e8_sb, e8_d)

            # persistent activations; kT/qT replicated at 4 partition bands
            # for 4-way row-tiled scores.
            kT_sb = cp.tile([128, SK], FP16)
            qT_sb = cp.tile([128, SQ], FP16)
            vT1_sb = cp.tile([9, SK], BF16)   # v rows 0-7, ones row 8
            vext_sb = cp.tile([128, 9 * 16], BF16)
            outU_sb = cp.tile([128, SQ], BF16)  # bands 0-8 & 64-72
            oes_sb = cp.tile([9, 512], BF16)    # oe reduce staging
            rden_sb = cp.tile([128, 16], F32)

            # row 8 of vT1 is the all-ones denominator row (engine writes
            # need 32-aligned partition bases, so fill it via DMA).
            nc.sync.dma_start(vT1_sb[8:9, :], vc_d)

            with tc.tile_pool(name="pxin", bufs=2) as xinp, \
                 tc.tile_pool(name="pcin", bufs=2) as cinp, \
                 tc.tile_pool(name="pkv", bufs=1, space="PSUM") as pkv, \
                 tc.tile_pool(name="psc", bufs=1, space="PSUM") as psc, \
                 tc.tile_pool(name="poe", bufs=1, space="PSUM") as poe, \
                 tc.tile_pool(name="pyp", bufs=2, space="PSUM") as pyp, \
                 tc.tile_pool(name="pet", bufs=2) as etp, \
                 tc.tile_pool(name="pys", bufs=2) as ysp:

                # ---- qT projection, all s-tiles up front (gates scores) --
                for st in range(4):
                    x_t = xinp.tile([128, 4096], FP16, tag="xt")
                    nc.sync.dma_start(
                        x_t.rearrange("p (c s) -> p c s", c=8),
                        xT[:, ts(st, 512)].rearrange("(c p) s -> p c s",
                                                     p=128),
                    )
                    # 2-way col tiling over dc parity: even dc sums at psum
                    # rows 0-7, odd at 64-71; combined during evacuation.
                    q_ps = poe.tile([128, 512], F32, tag="oe",
                                    name=f"q_{st}")
                    for dc2 in range(4):
                        nc.tensor.matmul(
                            q_ps[0:8, :], wq_sb[:, ds(72 * (2 * dc2), 8)],
                            x_t[:, ts(2 * dc2, 512)],
                            start=(dc2 == 0), stop=(dc2 == 3),
                            tile_position=(0, 0),
                        )
                        nc.tensor.matmul(
                            q_ps[64:72, :],
                            wq_sb[:, ds(72 * (2 * dc2 + 1) + 64, 8)],
                            x_t[:, ts(2 * dc2 + 1, 512)],
                            start=(dc2 == 0), stop=(dc2 == 3),
                            tile_position=(0, 64),
                        )
                    qs_sb = xinp.tile([8, 512], F32, tag="qs",
                                      name=f"qs_{st}")
                    nc.vector.tensor_copy(qs_sb, q_ps[64:72, :])
                    nc.vector.scalar_tensor_tensor(
                        qT_sb[0:8, ts(st, 512)], q_ps[0:8, :],
                        bq_sb[:, 0:1], qs_sb, ADD, ADD,
                    )
                    # replicate to bands 32/64/96 (engine-free)
                    for band in (32, 64, 96):
                        nc.sync.dma_start(qT_sb[ds(band, 8), ts(st, 512)],
                                          qT_sb[0:8, ts(st, 512)])

                # ---- main loop over s-tiles ------------------------------
                for st in range(4):
                    oe_ps = poe.tile([128, 512], F32, tag="oe",
                                       name=f"oe_{st}")
                    for g in range(4):
                        if st == 0:
                            # stream ctx / kv projection / v9 transposes
                            # for t-tile g; 2-way col tiling over dc parity.
                            ctx_t = cinp.tile([128, 4096], FP16, tag="ctx")
                            nc.sync.dma_start(
                                ctx_t.rearrange("p (c s) -> p c s", c=8),
                                ctxT[:, ts(g, 512)].rearrange(
                                    "(c p) s -> p c s", p=128),
                            )
                            kv_ps = pkv.tile([128, 512], F32, tag="kv",
                                             name=f"kv_{g}")
                            for dc2 in range(4):
                                nc.tensor.matmul(
                                    kv_ps[0:40, :],
                                    wkv_sb[:, ds(104 * (2 * dc2), 40)],
                                    ctx_t[:, ts(2 * dc2, 512)],
                                    start=(dc2 == 0), stop=(dc2 == 3),
                                    tile_position=(0, 0),
                                )
                                nc.tensor.matmul(
                                    kv_ps[64:104, :],
                                    wkv_sb[:, ds(104 * (2 * dc2 + 1) + 64,
                                                 40)],
                                    ctx_t[:, ts(2 * dc2 + 1, 512)],
                                    start=(dc2 == 0), stop=(dc2 == 3),
                                    tile_position=(0, 64),
                                )
                            # combine even/odd partials + bias during the
                            # evacuation (v rows 0-7/64-71, k rows 32-39/
                            # 96-103; all reads 32-aligned).
                            kvs_sb = cinp.tile([40, 512], F32, tag="kvs",
                                               name=f"kvs_{g}")
                            nc.vector.tensor_copy(kvs_sb, kv_ps[64:104, :])
                            nc.vector.scalar_tensor_tensor(
                                vT1_sb[0:8, ts(g, 512)], kv_ps[0:8, :],
                                bkv_sb[0:8, 0:1], kvs_sb[0:8, :], ADD, ADD,
                            )
                            nc.vector.scalar_tensor_tensor(
                                kT_sb[0:8, ts(g, 512)], kv_ps[32:40, :],
                                bkv_sb[32:40, 0:1], kvs_sb[32:40, :], ADD,
                                ADD,
                            )
                            for band in (32, 64, 96):
                                nc.sync.dma_start(
                                    kT_sb[ds(band, 8), ts(g, 512)],
                                    kT_sb[0:8, ts(g, 512)])
                            for c in range(4):
                                cc = 4 * g + c
                                tr_ps = pyp.tile([128, 9], BF16, tag="y",
                                                 name=f"tr_{g}_{c}")
                                nc.tensor.transpose(
                                    tr_ps, vT1_sb[0:9, ts(cc, 128)], id_sb)
                                nc.vector.tensor_copy(
                                    vext_sb[:, ds(9 * cc, 9)], tr_ps)

                        # scores group: 4 t-chunks concurrently (4-way row
                        # tiling) into a [128, 2048] PSUM group (4 banks).
                        sc_ps = psc.tile([128, 2048], F32, tag="sc",
                                         name=f"sc_{st}_{g}")
                        et = etp.tile([128, 2048], BF16, tag="et",
                                      name=f"et_{st}_{g}")
                        for i in range(4):
                            tcn = 4 * g + i
                            nc.tensor.matmul(
                                sc_ps[:, ts(i, 512)],
                                kT_sb[ds(32 * i, 8), ts(tcn, 128)],
                                qT_sb[ds(32 * i, 8), ts(st, 512)],
                                start=True, stop=True,
                                tile_position=(32 * i, 0),
                            )
                        nc.scalar.activation(et, sc_ps, EXP)
                        # oe: 2-way col tiling; even chunks accumulate at
                        # rows 0-8, odd chunks at rows 64-72.
                        for i in range(4):
                            tcn = 4 * g + i
                            half = i % 2
                            nc.tensor.matmul(
                                oe_ps[ds(64 * half, 9), :],
                                vext_sb[:, ds(9 * tcn, 9)],
                                et[:, ts(i, 512)],
                                start=(tcn < 2), stop=(tcn >= 14),
                                tile_position=(0, 64 * half),
                            )

                    # reduce oe partials into outU (band 0); band 64 copy
                    # for the 2-way y matmuls goes via DMA (engine-free).
                    nc.vector.tensor_copy(oes_sb, oe_ps[64:73, :])
                    nc.vector.tensor_add(outU_sb[0:9, ts(st, 512)],
                                         oe_ps[0:9, :], oes_sb)
                    nc.sync.dma_start(outU_sb[64:73, ts(st, 512)],
                                      outU_sb[0:9, ts(st, 512)])

                    # denominator, transposed to per-partition layout:
                    # dp[s, 1] = outU[0:9, chunk]^T @ e8
                    dp = pyp.tile([128, 4], F32, tag="y", name=f"dp_{st}")
                    for j in range(4):
                        nc.tensor.matmul(
                            dp[:, ds(j, 1)],
                            outU_sb[0:9, ds(512 * st + 128 * j, 128)],
                            e8_sb,
                            start=True, stop=True,
                        )
                    nc.vector.reciprocal(rden_sb[:, ts(st, 4)], dp[:, 0:4])

                    # y projection: 2-way row tiling over j pairs; the
                    # 1/den scaling doubles as the PSUM evacuation (24 on
                    # DVE, 8 on ACT to balance the engines).
                    for jp in range(2):
                        y_sbs = [ysp.tile([128, 1024], BF16, tag="ys",
                                          name=f"ysb_{st}_{jp}_{h2}")
                                 for h2 in range(2)]
                        for dh in range(2):
                            y_pss = [pyp.tile([128, 512], F32, tag="y",
                                              name=f"yps_{st}_{jp}_{dh}_{h2}")
                                     for h2 in range(2)]
                            for half in range(2):
                                j = 2 * jp + half
                                nc.tensor.matmul(
                                    y_pss[half],
                                    outU_sb[ds(64 * half, 9),
                                            ds(512 * st + 128 * j, 128)],
                                    wob_sb[ds(64 * half, 9), ts(dh, 512)],
                                    start=True, stop=True,
                                    tile_position=(64 * half, 0),
                                )
                            for half in range(2):
                                j = 2 * jp + half
                                if jp == 1 and half == 1:
                                    nc.scalar.activation(
                                        y_sbs[half][:, ts(dh, 512)],
                                        y_pss[half], CPY, bias=0.0,
                                        scale=rden_sb[:, ds(4 * st + j, 1)],
                                    )
                                else:
                                    nc.vector.tensor_scalar_mul(
                                        y_sbs[half][:, ts(dh, 512)],
                                        y_pss[half],
                                        rden_sb[:, ds(4 * st + j, 1)],
                                    )
                        for half in range(2):
                            j = 2 * jp + half
                            nc.sync.dma_start(
                                y_d[ds(128 * (4 * st + j), 128), :],
                                y_sbs[half],
                            )

    nc.compile()
    return nc


def _get_nc():
    if "nc" not in _CACHE:
        _CACHE["nc"] = _build_nc()
    return _CACHE["nc"]


def _bf16(x):
    import ml_dtypes
    return np.asarray(x, np.float32).astype(ml_dtypes.bfloat16)


def _fp16(x):
    return np.asarray(x, np.float32).astype(np.float16)


def _prep_params(Wq, bq, Wk, bk, Wv, bv, Wo, bo):
    f32 = np.float32
    Wq = np.asarray(Wq, f32)
    Wk = np.asarray(Wk, f32)
    Wv = np.asarray(Wv, f32)
    Wo = np.asarray(Wo, f32)
    # q: per-dc [128, 72] block, Wq_dc at cols 0-7 and duplicated at 64-71
    wq = np.zeros((8, 128, 72), f32)
    wq[:, :, 0:8] = Wq.reshape(8, 128, 8)
    wq[:, :, 64:72] = Wq.reshape(8, 128, 8)
    wq_l = _fp16(np.ascontiguousarray(
        wq.transpose(1, 0, 2).reshape(128, 8 * 72)))
    # kv: per-dc [128, 104] block: v cols 0-7, k cols 32-39, dup at +64
    wkv = np.zeros((8, 128, 104), f32)
    wkv[:, :, 0:8] = Wv.reshape(8, 128, 8)
    wkv[:, :, 32:40] = Wk.reshape(8, 128, 8)
    wkv[:, :, 64:72] = Wv.reshape(8, 128, 8)
    wkv[:, :, 96:104] = Wk.reshape(8, 128, 8)
    wkv_l = _fp16(np.ascontiguousarray(
        wkv.transpose(1, 0, 2).reshape(128, 8 * 104)))
    wob9 = np.concatenate([Wo, np.asarray(bo, f32)[None, :]], axis=0)
    wob = np.zeros((128, D), f32)
    wob[0:9] = wob9
    wob[64:73] = wob9
    e8 = np.zeros((9, 1), f32)
    e8[8, 0] = 1.0
    bkv = np.zeros((128, 1), f32)
    bkv[0:8, 0] = np.asarray(bv, f32)
    bkv[32:40, 0] = np.asarray(bk, f32)
    return {
        "wq_l": wq_l, "wkv_l": wkv_l,
        "bq8": np.asarray(bq, f32).reshape(8, 1),
        "bkv": bkv,
        "wob": _bf16(wob), "ident9": _bf16(np.eye(9, dtype=f32)),
        "vconst": _bf16(np.ones((1, SK), f32)), "e8": _bf16(e8),
    }


def make_in_maps(x, context, Wq, bq, Wk, bk, Wv, bv, Wo, bo):
    f32 = np.float32
    x = np.asarray(x, f32)
    context = np.asarray(context, f32)
    xT = _fp16(np.ascontiguousarray(x.transpose(0, 2, 1)))  # [B, D, SQ]
    ctxT = _fp16(np.ascontiguousarray(context.transpose(0, 2, 1)))
    params = _prep_params(Wq, bq, Wk, bk, Wv, bv, Wo, bo)
    return [
        {"xT": xT[b], "ctxT": ctxT[b], **params} for b in range(N_CORES)
    ]


def kernel(x, context, Wq, bq, Wk, bk, Wv, bv, Wo, bo):
    import concourse.bass_utils as bass_utils

    nc = _get_nc()
    in_maps = make_in_maps(x, context, Wq, bq, Wk, bk, Wv, bv, Wo, bo)
    res = bass_utils.run_bass_kernel_spmd(
        nc, in_maps, core_ids=list(range(N_CORES)))
    return np.stack(
        [np.asarray(res.results[b]["y"]).astype(np.float32)
         for b in range(N_CORES)], axis=0)
